# revision 7
# baseline (speedup 1.0000x reference)
"""Trainium2 Bass kernel for nn_AgentGnn_CRAT (2-layer CGConv GNN).

Structure exploited: the graph is B=1024 independent fully-connected
16-agent cliques (no self loops).  For edge (s -> t) within a sample:

    z = [x_t, x_s, c_t - c_s]                       (258 dims)
    m = sigmoid(z @ wf.T + bf) * softplus(z @ ws.T + bs)
    agg[t] = sum_{s != t} m(s, t)
    out = relu(batchnorm(agg) + x)                  (x2 layers)

Since z @ wf.T splits into a target part and a source part,
    a_f(s,t) = P_f[t] + Q_f[s]
      P_f = Wt_f^T x + Wc_f^T c + bf     (per node)
      Q_f = Ws_f^T x - Wc_f^T c          (per node)
so the per-edge work is a broadcast add of per-node vectors, done
dense over all 16x16 pairs per sample.

The compiler's ACT tables have no Softplus, and sigmoid would need a
DVE reciprocal.  Instead:
    2 * sigmoid(a) = tanh(a/2) + 1       (tanh + exp share one table)
    softplus(b) = ln(exp(b) + 1)         (ln in another table)
    m' = (tanh(a/2) + 1) * softplus(b) = 2 * m
and the constant factor 2 is absorbed exactly by the train-mode
batchnorm that follows the aggregation.  The f-gate weights/bias are
host-halved so the broadcast add directly yields a/2; the diagonal
(s==t) is memset to -30 before tanh so (tanh+1) ~ 1e-26 and those
messages vanish from the aggregation.
    rsqrt(v) = exp(-0.5 * ln(v))          (for batchnorm)

Sharding: data parallel over samples -- each of 8 cores gets 2048
nodes (128 samples).  BatchNorm batch stats are combined with a tiny
[128,3] AllReduce (mean, var, mean^2 per feature).

Layout: features (H=128) on partitions, nodes/pairs on the free axis.
Host pre-transposes inputs and post-transposes the output.
"""

import numpy as np

H = 128          # latent dim = partition dim
D = 2            # edge attr dim
A = 16           # agents per sample
B = 1024         # samples
N = B * A        # 16384 nodes
N_CORES = 8
NL = N // N_CORES        # 2048 nodes per core
SL = NL // A             # 128 samples per core
CS = 8                   # samples per phase-2 chunk
PC = CS * A * A          # 2048 pair columns per chunk
NCH = SL // CS           # 16 chunks
BLK = 512                # matmul free-dim block (one PSUM bank of f32)
EPS = 1e-5
DIAG_KILL = -30.0        # tanh(-30) = -1 + ~1e-26 -> diagonal message ~ 0
TGROUP = 2               # chunks per ACT-table batch (amortizes table loads)

_CACHE = {}


# --------------------------------------------------------------------------
# bass program
# --------------------------------------------------------------------------

def _build_bass():
    from concourse import bass, bacc, tile, mybir

    f32 = mybir.dt.float32
    AF = mybir.ActivationFunctionType
    OP = mybir.AluOpType

    nc = bacc.Bacc("TRN2", target_bir_lowering=False, debug=False,
                   num_devices=N_CORES)

    xT = nc.dram_tensor("xT", [H, NL], f32, kind="ExternalInput").ap()
    cT = nc.dram_tensor("cT", [D, NL], f32, kind="ExternalInput").ap()
    # 8 blocks of [128,128] lhsT weights: per layer (wt_f/2, ws_f/2, wt_s, ws_s)
    Wd = nc.dram_tensor("W", [H, 8 * H], f32, kind="ExternalInput").ap()
    # 8 blocks of [2,128]: per layer (wc_f/2, -wc_f/2, wc_s, -wc_s)
    WCd = nc.dram_tensor("WC", [D, 8 * H], f32, kind="ExternalInput").ap()
    # per-feature vectors: cols = (bf/2, bs, gamma, beta) x 2 layers
    Vd = nc.dram_tensor("V", [H, 8], f32, kind="ExternalInput").ap()
    outT = nc.dram_tensor("outT", [H, NL], f32, kind="ExternalOutput").ap()

    with tile.TileContext(nc) as tc:
        with (
            tc.tile_pool(name="res", bufs=1) as res,
            tc.tile_pool(name="pq", bufs=1) as pqp,
            tc.tile_pool(name="ch", bufs=2) as ch,
            tc.tile_pool(name="psum", bufs=2, space="PSUM") as psp,
            tc.tile_pool(name="dram", bufs=1, space="DRAM") as dram,
        ):
            x0 = res.tile([H, NL], f32, tag="x0", name="x0")
            c = res.tile([D, NL], f32, tag="c", name="c")
            w = res.tile([H, 8 * H], f32, tag="w", name="w")
            wc = res.tile([D, 8 * H], f32, tag="wc", name="wc")
            v = res.tile([H, 8], f32, tag="v", name="v")
            nc.sync.dma_start(x0[:], xT[:])
            nc.sync.dma_start(c[:], cT[:])
            nc.sync.dma_start(w[:], Wd[:])
            nc.sync.dma_start(wc[:], WCd[:])
            nc.sync.dma_start(v[:], Vd[:])

            x_in = x0
            for l in range(2):
                # ------------- phase 1: per-node P/Q matmuls -------------
                # produces negPf, negQf (f gate, negated) and Ps, Qs
                Pf = pqp.tile([H, NL], f32, tag="Pf", name=f"Pf{l}")
                Qf = pqp.tile([H, NL], f32, tag="Qf", name=f"Qf{l}")
                Ps = pqp.tile([H, NL], f32, tag="Ps", name=f"Ps{l}")
                Qs = pqp.tile([H, NL], f32, tag="Qs", name=f"Qs{l}")
                for blk in range(NL // BLK):
                    sl = slice(blk * BLK, (blk + 1) * BLK)
                    for g, (Pt, Qt) in enumerate(((Pf, Qf), (Ps, Qs))):
                        wb = l * 4 * H + g * 2 * H
                        ps1 = psp.tile([H, BLK], f32, tag="psP",
                                       name=f"psP{l}_{blk}_{g}")
                        nc.tensor.matmul(ps1[:], w[:, wb:wb + H], x_in[:, sl],
                                         start=True, stop=False)
                        nc.tensor.matmul(ps1[:], wc[:, wb:wb + H], c[:, sl],
                                         start=False, stop=True)
                        nc.scalar.activation(Pt[:, sl], ps1[:], AF.Identity,
                                             bias=v[:, l * 4 + g:l * 4 + g + 1])
                        ps2 = psp.tile([H, BLK], f32, tag="psQ",
                                       name=f"psQ{l}_{blk}_{g}")
                        nc.tensor.matmul(ps2[:], w[:, wb + H:wb + 2 * H],
                                         x_in[:, sl], start=True, stop=False)
                        nc.tensor.matmul(ps2[:], wc[:, wb + H:wb + 2 * H],
                                         c[:, sl], start=False, stop=True)
                        nc.vector.tensor_copy(Qt[:, sl], ps2[:])

                # ------------- phase 2: pair stage -----------------------
                agg = pqp.tile([H, NL], f32, tag="agg", name=f"agg{l}")
                stats = res.tile([H, NCH * 6], f32, tag="stats",
                                 name=f"stats{l}")

                def pair_view(src, ci, is_target):
                    ncols = slice(ci * CS * A, (ci + 1) * CS * A)
                    return (src[:, ncols]
                            .rearrange("p (b t) -> p b t", b=CS)
                            .unsqueeze(3 if is_target else 2)
                            .broadcast_to([H, CS, A, A]))

                for cg in range(NCH // TGROUP):
                    group = [cg * TGROUP + k for k in range(TGROUP)]
                    a2s, bts, Gs, Us, SPs = {}, {}, {}, {}, {}
                    for ci in group:
                        a2 = ch.tile([H, PC], f32, tag="a2",
                                     name=f"a2_{l}_{ci}")
                        a2s[ci] = a2
                        a24 = a2[:].rearrange("p (b t s) -> p b t s",
                                              b=CS, t=A)
                        # a/2 = Pf[t] + Qf[s]   (f weights host-halved)
                        nc.vector.tensor_tensor(a24, pair_view(Pf, ci, True),
                                                pair_view(Qf, ci, False),
                                                op=OP.add)
                        # kill diagonal (s==t): tanh -> -1, gate -> 0
                        diag = (a2[:].rearrange("p (b q) -> p b q", b=CS)
                                [:, :, 0:A * A:A + 1])
                        nc.gpsimd.memset(diag, DIAG_KILL)
                        bt = ch.tile([H, PC], f32, tag="bt",
                                     name=f"bt_{l}_{ci}")
                        bts[ci] = bt
                        bt4 = bt[:].rearrange("p (b t s) -> p b t s",
                                              b=CS, t=A)
                        nc.gpsimd.tensor_tensor(bt4, pair_view(Ps, ci, True),
                                                pair_view(Qs, ci, False),
                                                op=OP.add)
                    # table A (exp_and_others): tanh + exp, batched
                    for ci in group:
                        G = ch.tile([H, PC], f32, tag="G", name=f"G_{l}_{ci}")
                        Gs[ci] = G
                        nc.scalar.activation(G[:], a2s[ci][:], AF.Tanh)
                        U = ch.tile([H, PC], f32, tag="U", name=f"U_{l}_{ci}")
                        Us[ci] = U
                        nc.scalar.activation(U[:], bts[ci][:], AF.Exp)
                    # table B (natural_log): softplus = ln(U + 1), batched
                    for ci in group:
                        SP = ch.tile([H, PC], f32, tag="SP",
                                     name=f"SP_{l}_{ci}")
                        SPs[ci] = SP
                        nc.scalar.activation(SP[:], Us[ci][:], AF.Ln,
                                             bias=1.0)
                    for ci in group:
                        ncols = slice(ci * CS * A, (ci + 1) * CS * A)
                        # m' = (tanh(a/2) + 1) * softplus(b)  (= 2m, BN
                        # absorbs the factor); overwrite U
                        m = Us[ci][:]
                        nc.vector.scalar_tensor_tensor(
                            m, Gs[ci][:], 1.0, SPs[ci][:],
                            op0=OP.add, op1=OP.mult)
                        nc.vector.tensor_reduce(
                            agg[:, ncols],
                            m.rearrange("p (n s) -> p n s", s=A),
                            axis=mybir.AxisListType.X, op=OP.add)
                        nc.vector.bn_stats(stats[:, ci * 6:(ci + 1) * 6],
                                           agg[:, ncols])

                # ------------- phase 3: BN + residual + relu -------------
                pack = res.tile([H, 4], f32, tag="pack", name=f"pack{l}")
                nc.vector.bn_aggr(pack[:, 0:2], stats[:])
                nc.scalar.activation(pack[:, 2:3], pack[:, 0:1], AF.Square)

                cin = dram.tile([H, 3], f32, tag=f"cin{l}", name=f"cin{l}")
                cout = dram.tile([H, 3], f32, tag=f"cout{l}", name=f"cout{l}")
                nc.sync.dma_start(cin[:], pack[:, 0:3])
                nc.gpsimd.collective_compute(
                    "AllReduce", OP.add,
                    ins=[cin.opt()], outs=[cout.opt()],
                    replica_groups=[list(range(N_CORES))])
                red = res.tile([H, 3], f32, tag="red", name=f"red{l}")
                nc.sync.dma_start(red[:], cout[:])

                bnp = res.tile([H, 12], f32, tag="bnp", name=f"bnp{l}")
                (mg, ex2t, ex2, msq, var, vare, lnv, inv, sca, tb,
                 bia) = (bnp[:, i:i + 1] for i in range(11))
                nc.vector.tensor_scalar_mul(mg, red[:, 0:1], 1.0 / N_CORES)
                nc.vector.tensor_tensor(ex2t, red[:, 1:2], red[:, 2:3],
                                        op=OP.add)
                nc.vector.tensor_scalar_mul(ex2, ex2t, 1.0 / N_CORES)
                nc.vector.tensor_tensor(msq, mg, mg, op=OP.mult)
                nc.vector.tensor_tensor(var, ex2, msq, op=OP.subtract)
                nc.vector.tensor_scalar_add(vare, var, EPS)
                # rsqrt via the exp/ln table: exp(-0.5 * ln(v))
                nc.scalar.activation(lnv, vare, AF.Ln)
                nc.scalar.activation(inv, lnv, AF.Exp, scale=-0.5)
                nc.vector.tensor_tensor(sca, inv, v[:, l * 4 + 2:l * 4 + 3],
                                        op=OP.mult)
                nc.vector.tensor_tensor(tb, mg, sca, op=OP.mult)
                nc.vector.tensor_tensor(bia, v[:, l * 4 + 3:l * 4 + 4], tb,
                                        op=OP.subtract)

                y = pqp.tile([H, NL], f32, tag="y", name=f"y{l}")
                nc.vector.scalar_tensor_tensor(y[:], agg[:], sca, x_in[:],
                                               op0=OP.mult, op1=OP.add)
                xn = res.tile([H, NL], f32, tag=f"x{l + 1}", name=f"x{l + 1}")
                nc.scalar.activation(xn[:], y[:], AF.Relu, bias=bia)
                x_in = xn

            nc.sync.dma_start(outT[:], x_in[:])

    nc.compile()
    return nc


def get_nc():
    if "nc" not in _CACHE:
        _CACHE["nc"] = _build_bass()
    return _CACHE["nc"]


# --------------------------------------------------------------------------
# host-side sharding / packing
# --------------------------------------------------------------------------

def prep_in_maps(gnn_in, centers, wf1, bf1, ws1, bs1, g1, be1,
                 wf2, bf2, ws2, bs2, g2, be2):
    blocks_w, blocks_wc, cols_v = [], [], []
    for wf, bf, ws, bs, gm, be in ((wf1, bf1, ws1, bs1, g1, be1),
                                   (wf2, bf2, ws2, bs2, g2, be2)):
        # f gate (halved, for tanh(a/2)), then s gate
        for mat, sc in ((wf, 0.5), (ws, 1.0)):
            blocks_w.append(sc * mat[:, :H].T)             # wt
            blocks_w.append(sc * mat[:, H:2 * H].T)        # ws
            wce = mat[:, 2 * H:2 * H + D].T                # [2,128]
            blocks_wc.append(sc * wce)                     # used in P
            blocks_wc.append(-sc * wce)                    # used in Q
        cols_v += [0.5 * bf, bs, gm, be]
    W = np.ascontiguousarray(np.concatenate(blocks_w, axis=1),
                             dtype=np.float32)             # [128,1024]
    WC = np.ascontiguousarray(np.concatenate(blocks_wc, axis=1),
                              dtype=np.float32)            # [2,1024]
    V = np.ascontiguousarray(np.stack(cols_v, axis=1), dtype=np.float32)

    in_maps = []
    for cid in range(N_CORES):
        rows = slice(cid * NL, (cid + 1) * NL)
        in_maps.append({
            "xT": np.ascontiguousarray(gnn_in[rows].T, dtype=np.float32),
            "cT": np.ascontiguousarray(centers[rows].T, dtype=np.float32),
            "W": W, "WC": WC, "V": V,
        })
    return in_maps


def _canonical_edge_index():
    i, j = np.meshgrid(np.arange(A), np.arange(A), indexing="ij")
    mask = i != j
    li, lj = i[mask], j[mask]
    offs = (np.arange(B) * A)[:, None]
    rows = (li[None, :] + offs).reshape(-1)
    cols = (lj[None, :] + offs).reshape(-1)
    return np.stack([rows, cols])


def _numpy_fallback(gnn_in, centers, edge_index, params):
    """Generic (slow) host implementation for non-canonical edge_index."""
    row, col = np.asarray(edge_index[0]), np.asarray(edge_index[1])
    eattr = centers[col] - centers[row]
    x = gnn_in

    def softplus(z):
        return np.maximum(z, 0.0) + np.log1p(np.exp(-np.abs(z)))

    def cgconv(x, wf, bf, ws, bs, gm, be):
        z = np.concatenate([x[col], x[row], eattr], axis=-1)
        mf = 1.0 / (1.0 + np.exp(-(z @ wf.T + bf)))
        m = mf * softplus(z @ ws.T + bs)
        agg = np.zeros_like(x)
        np.add.at(agg, col, m)
        mean = agg.mean(axis=0)
        var = agg.var(axis=0)
        bn = (agg - mean) / np.sqrt(var + EPS) * gm + be
        return bn + x

    x = np.maximum(cgconv(x, *params[0]), 0.0)
    x = np.maximum(cgconv(x, *params[1]), 0.0)
    return x.astype(np.float32)


# --------------------------------------------------------------------------
# entry point
# --------------------------------------------------------------------------

def kernel(gnn_in, centers, edge_index, wf1, bf1, ws1, bs1, g1, be1,
           wf2, bf2, ws2, bs2, g2, be2):
    gnn_in = np.asarray(gnn_in, dtype=np.float32)
    centers = np.asarray(centers, dtype=np.float32)
    args = [np.asarray(a, dtype=np.float32)
            for a in (wf1, bf1, ws1, bs1, g1, be1,
                      wf2, bf2, ws2, bs2, g2, be2)]

    ei = np.asarray(edge_index)
    if ei.shape != (2, B * A * (A - 1)) or \
            not np.array_equal(ei, _canonical_edge_index()):
        return _numpy_fallback(gnn_in, centers, ei,
                               (tuple(args[0:6]), tuple(args[6:12])))

    from concourse import bass_utils
    nc = get_nc()
    in_maps = prep_in_maps(gnn_in, centers, *args)
    res = bass_utils.run_bass_kernel_spmd(nc, in_maps,
                                          core_ids=list(range(N_CORES)))
    out = np.empty((N, H), dtype=np.float32)
    for cid in range(N_CORES):
        out[cid * NL:(cid + 1) * NL] = res.results[cid]["outT"].T
    return out
